# revision 36
# baseline (speedup 1.0000x reference)
"""Trainium2 Bass kernel for masked similar-user attention.

Computation (per batch b, position s):
    scores[u] = dot(user[b], sim[b,s,u,:])        (u = 50 similar users, d = 32)
    scores    = where(mask, -1e9, scores)
    attn      = softmax(scores)
    out[s]    = sum_u attn[u] * sim[b,s,u,:] + item[b,s]

Sharding: pure data parallel over batch (B=512 -> 64 per core, 8 cores).

v4 changes (v2 = 502us, v3 = 505us):
  * mul2 (sim * e broadcast) ran in DVE 1x mode in v2 because its
    broadcast operand had innermost stride 0 (2x_1p requires stride +-1
    on every operand): 1883ns vs 1040ns for the same-size mul1. Fix:
    ACT writes exp twice into dense pairs ec2[P,G*U,2]; phase_b expands
    to e32[P,G*U,32] with 5 single-src 4x-mode DVE copies (~2.4us),
    then mul2 is all-stride-1 -> 2x. (v3 built e32 on ACT with strided
    writes + big copies: ACT hit 50% busy and the SBUF contention
    inflated EVERY DVE op by 300-600ns -- a complete wash at 505us.
    Keep ACT cheap and dense.)
  * mask folded into one sim column host-side (d0 = argmax|user|,
    sim[..,d0] += -1e9/user[d0]): kills the per-tile mask add and
    shrinks the packed row 1714 -> 1664 words.
  * G=5 (640-row tiles, NT=20): fewer per-tile fixed costs for the
    fold-tree/recip/small ops (~300ns fixed per DVE op).
  * reciprocal_approx_fast was tried for the [P,G] denominators:
    "ISA wrong length" from this walrus build; reverted to 4x [P,1]
    v.reciprocal.

v2 implementation notes (v1 = plain f32 tensor_tensor/tensor_reduce, 848us;
this version measures ~502us, DVE-bound at ~95% busy):
  * All operands packed host-side into ONE bf16 row-major DRAM tensor
    [sim(1600) | user(32) | maskf(50) | item(32)] -> 3428B/row, halving HBM
    traffic and enabling the DVE 2-byte (2x_1p) fast mode: all-bf16
    tensor_tensor ops with packed innermost dims run at 0.5 cyc/elem
    (measured: 1600-elem multiply = 832ns vs 1667ns in f32).
  * Reductions are fold trees of in-place tensor_adds (contiguous, 2x)
    instead of tensor_reduce (no fast mode, and the transposed u-reduce
    paid an extra 1.64x stride penalty in v1). The u=50 fold uses the flat
    (u d) view: 50->25->16(tail 9 into 7:16)->8->4->2->1, all 3-dim APs
    (walrus rejects 4-dim elementwise APs).
  * exp+accum on the otherwise-idle ACT engine (compact [P,50] per group;
    broadcast-input activation reads produce garbage on HW - do not expand).
  * v.reciprocal only behaves for [P,1] shapes; per-group.
  * Ops are tile-granular (128 partitions x 4 row-groups) to amortize the
    ~70ns fixed DVE instruction cost; 25 tiles of 512 rows per core.
  * Loads AND stores on the SP queue (one shared VB wait per tile).
  * GpSimd offload was tried and reverted: Q7 software tensor ops ran far
    below the 0.42-efficiency model and stalled DVE (643us vs 502us).

Pipeline (per tile T; sems LD/ST/VA/AS/VB):
    SP : prologue loads 0-3; iter T: [wait VB>=T+1] store T, load T+4
    DVE: iter T: [wait LD] A(T): mul1 x4, fold_d x5 -> scores, +mask .inc VA
         [wait AS>=T] [wait ST] B(T-1): recip x4, mul2 x4, fold_u x6,
                                stt x4 -> outt  .inc VB
    ACT: iter T: [wait VA>=T+1] 4x exp(g, accum esum)  .inc AS
"""

import sys

if "/opt/trn_rl_repo" not in sys.path:
    sys.path.insert(0, "/opt/trn_rl_repo")

import numpy as np
import ml_dtypes

import concourse.bass as bass
from concourse import mybir
from concourse.bass_utils import run_bass_kernel_spmd


def _install_ntff_hook_shim():
    """The container's antenv lacks axon_hooks; recreate it so
    run_bass_kernel_spmd(trace=True) can capture NTFF profiles through
    libaxon_pjrt.so (same ctypes path trn_boot uses)."""
    import contextlib
    import ctypes
    import types

    if "antenv.axon_hooks" in sys.modules:
        return
    so_path = "/opt/axon/libaxon_pjrt.so"
    try:
        lib = ctypes.CDLL(so_path)
    except OSError:
        return
    if not hasattr(lib, "axon_start_nrt_profile"):
        return
    lib.axon_start_nrt_profile.argtypes = [
        ctypes.POINTER(ctypes.c_int64),
        ctypes.c_size_t,
    ]
    lib.axon_start_nrt_profile.restype = ctypes.c_int64
    lib.axon_stop_nrt_profile.argtypes = [ctypes.c_char_p]
    lib.axon_stop_nrt_profile.restype = ctypes.c_int64

    @contextlib.contextmanager
    def _hook(output_dir, device_ids):
        import jax

        jax.devices()
        if device_ids:
            ids = (ctypes.c_int64 * len(device_ids))(*device_ids)
            rc = lib.axon_start_nrt_profile(ids, len(device_ids))
        else:
            rc = lib.axon_start_nrt_profile(None, 0)
        if rc != 0:
            raise RuntimeError(f"axon_start_nrt_profile rc={rc}")
        try:
            yield
        finally:
            n = lib.axon_stop_nrt_profile(str(output_dir).encode())
            print(f"ntff profile: {n} file(s) written to {output_dir}")

    mod = types.ModuleType("antenv.axon_hooks")
    mod.get_axon_ntff_profile_hook = lambda: _hook
    mod.set_axon_ntff_profile_hook = lambda h: None
    sys.modules["antenv.axon_hooks"] = mod


_install_ntff_hook_shim()

# ---------------------------------------------------------------- config
B, S, U, D = 512, 200, 50, 32
NCORES = 8
BC = B // NCORES            # batches per core = 64
ROWS = BC * S               # rows per core = 12800
P = 128                     # SBUF partitions
G = 10                      # row-groups of 128 per DMA tile
NT = ROWS // (P * G)        # outer tiles per core = 10
NEG = -1e9
NPK = 2                     # pkt ring depth (tiles are 2.1x bigger at G=10)

UD = U * D                  # 1600
ROWW = UD + D + D           # packed row width (bf16 words) = 1664 (mask folded into sim)
o_user, o_item = UD, UD + D
# tile-block layout: per (tile, partition) the DRAM holds
#   [ sim for g=0..G-1 (G*1600) | user g=0..G-1 (G*32) | item g=0..G-1 (G*32) ]
# so the G sim groups are one contiguous [P, G*U, D] region -> mul2/item ops
# span all groups in ONE instruction (per-op fixed cost is ~290ns; v4 spent
# ~40% of DVE time on 36 ops/tile of overhead).
BLKW = G * ROWW             # per-partition tile block words = 8320
o_ub = G * UD               # user block offset = 8000
o_ib = G * UD + G * D       # item block offset = 8160

BF16 = np.dtype(ml_dtypes.bfloat16)


def _audit_waits(nc, max_waits=1):
    bad = []
    for blk in nc.m.functions[0].blocks:
        for ins in blk.instructions:
            si = ins.sync_info
            if si is not None and len(si.on_wait) > max_waits:
                bad.append((blk.name, ins.name, ins.opcode, len(si.on_wait)))
    if bad:
        msg = "\n".join(f"  {b}/{n} {o}: {k} waits" for b, n, o, k in bad)
        raise RuntimeError(f"instructions exceeding {max_waits} sync wait(s):\n{msg}")


# ---------------------------------------------------------------- kernel IR
def _build_nc():
    f32 = mybir.dt.float32
    bf16 = mybir.dt.bfloat16
    MUL = mybir.AluOpType.mult
    ADD = mybir.AluOpType.add
    nc = bass.Bass()

    pk_d = nc.dram_tensor("pk", [NT * P, BLKW], bf16, kind="ExternalInput")
    out_d = nc.dram_tensor("out", [ROWS, D], f32, kind="ExternalOutput")

    pk_v = pk_d[:].rearrange("(T p) f -> T p f", p=P)
    out_v = out_d[:].rearrange("(T g p) f -> T p g f", g=G, p=P)

    # SBUF buffers (all elementwise-op APs kept <= 3 dims: partition + 2 free)
    pkt = [nc.alloc_sbuf_tensor(f"pkt{i}", [P, BLKW], bf16) for i in range(NPK)]
    tmp = nc.alloc_sbuf_tensor("tmp", [P, G * U, D], bf16)    # mul1 out + fold_d scratch
    tmp2 = nc.alloc_sbuf_tensor("tmp2", [P, G * U, D], bf16)  # mul2 out + fold_u scratch
    scores = [nc.alloc_sbuf_tensor(f"scores{i}", [P, G * U], f32) for i in range(2)]
    # ec2[., u, 0:2] = exp(scores[., u]) written twice by ACT (dense pairs);
    # phase_b log2-doubles it into e32[., u, 0:32] with DVE 4x-mode copies so
    # that mul2's weight operand has innermost stride 1 -> 2x_1p mode
    # (a stride-0 broadcast operand forces 1x: 1883 vs 1040ns measured).
    # v3 built e32 on ACT instead: the strided ACT traffic inflated every
    # DVE op by 300-600ns (SBUF contention), a complete wash -- keep ACT light.
    ec2 = [nc.alloc_sbuf_tensor(f"ec2_{i}", [P, G * U, 2], bf16) for i in range(2)]
    e8 = [nc.alloc_sbuf_tensor(f"e8_{i}", [P, G * U, D // 4], bf16) for i in range(2)]
    esum = [nc.alloc_sbuf_tensor(f"esum{i}", [P, G], f32) for i in range(2)]
    lnes = nc.alloc_sbuf_tensor("lnes", [P, G], f32)
    recip = [nc.alloc_sbuf_tensor(f"recip{i}", [P, G], f32) for i in range(2)]
    outw = nc.alloc_sbuf_tensor("outw", [P, G, D], f32)
    outt = [nc.alloc_sbuf_tensor(f"outt{i}", [P, G * D], f32) for i in range(2)]

    LD = nc.alloc_semaphore("LD")
    ST = nc.alloc_semaphore("ST")
    VA = nc.alloc_semaphore("VA")
    AS = nc.alloc_semaphore("AS")
    VB = nc.alloc_semaphore("VB")


    def views(T):
        pkb = pkt[T % NPK][:]                                            # [P, BLKW]
        simblk = pkb[:, :o_ub].rearrange("p (u d) -> p u d", d=D)        # [P, G*U, D]
        usert = pkb[:, o_ub:o_ib].rearrange("p (g d) -> p g d", d=D)     # [P, G, D]
        itemt = pkb[:, o_ib:]                                            # [P, G*D]
        return simblk, usert, itemt

    with nc.Block() as blk:

        @blk.sync
        def _(sp):
            for T in range(min(NPK, NT)):
                sp.dma_start(out=pkt[T][:], in_=pk_v[T]).then_inc(LD, 16)
            for T in range(NT):
                sp.wait_ge(VB, T + 1)
                sp.dma_start(
                    out=out_v[T],
                    in_=outt[T % 2][:].rearrange("p (g w) -> p g w", g=G),
                ).then_inc(ST, 16)
                if T + NPK < NT:
                    sp.dma_start(
                        out=pkt[(T + NPK) % NPK][:], in_=pk_v[T + NPK]
                    ).then_inc(LD, 16)

        def phase_a(v, T):
            simblk, usert, _ = views(T)
            sc = scores[T % 2][:]
            # products (all-bf16 tensor_tensor -> 2x mode), per group (3-dim APs)
            for g in range(G):
                sim3 = simblk[:, g * U : (g + 1) * U, :]
                ub = usert[:, g, :].unsqueeze(1).broadcast_to([P, U, D])
                v.tensor_mul(tmp[:, g * U : (g + 1) * U, :], sim3, ub)
            # fold d: 32 -> 16 -> 8 -> 4 -> 2 (in place, all-bf16 2x)
            for k in (16, 8, 4, 2):
                v.tensor_add(tmp[:, :, 0:k], tmp[:, :, 0:k], tmp[:, :, k : 2 * k])
            # 2 -> 1, f32 out. The -1e9 mask is pre-folded into one sim column
            # host-side (sim[.,.,u,d0] += maskNEG/user[d0], d0 = argmax|user|),
            # so the dot product yields scores+mask directly -- no mask op.
            v.tensor_add(sc, tmp[:, :, 0], tmp[:, :, 1]).then_inc(VA, 1)

        def phase_b(v, T):
            simblk, _, itemt = views(T)
            et = e8[T % 2][:]
            # weighted values: tmp2 = sim * e (all-bf16, stride-1 -> 2x),
            # one op per d-quarter across ALL groups (sim groups contiguous)
            for j in range(4):
                v.tensor_mul(
                    tmp2[:, :, 8 * j : 8 * j + 8], simblk[:, :, 8 * j : 8 * j + 8], et
                )
            # fold u via the flat (u d) view: 50 -> 25 -> 16 -> 8 -> 4 -> 2 -> 1
            t2 = tmp2[:].rearrange("p (g u) d -> p g (u d)", g=G)
            v.tensor_add(t2[:, :, 0:800], t2[:, :, 0:800], t2[:, :, 800:1600])
            v.tensor_add(t2[:, :, 224:512], t2[:, :, 224:512], t2[:, :, 512:800])
            for k in (256, 128, 64):
                v.tensor_add(t2[:, :, 0:k], t2[:, :, 0:k], t2[:, :, k : 2 * k])
            v.tensor_add(outw[:], t2[:, :, 0:32], t2[:, :, 32:64])
            # out = outw * (1/esum) + item: recip comes from ACT as
            # exp(-ln(esum)) so the tail is 2 whole-tile ops instead of
            # 5x [P,1] reciprocal + 5x per-group stt (AluOpType.divide in
            # tensor_tensor fails 's3s3d3_tt_valid_op').
            rx = recip[T % 2][:].unsqueeze(2).broadcast_to([P, G, D])
            ot = outt[T % 2][:]
            v.tensor_mul(ot.rearrange("p (g w) -> p g w", g=G), outw[:], rx)
            ins = v.tensor_add(ot, ot, itemt)
            ins.then_inc(VB, 1)

        @blk.vector
        def _(v):
            for T in range(NT):
                v.wait_ge(LD, 16 * (T + 1))
                phase_a(v, T)
                if T >= 1:
                    v.wait_ge(AS, T)
                    if T >= 3:
                        v.wait_ge(ST, 16 * (T - 2))
                    phase_b(v, T - 1)
            v.wait_ge(AS, NT)
            v.wait_ge(ST, 16 * (NT - 2))
            phase_b(v, NT - 1)

        @blk.scalar
        def _(a):
            Exp = mybir.ActivationFunctionType.Exp
            Copy = mybir.ActivationFunctionType.Copy
            for T in range(NT):
                a.wait_ge(VA, T + 1)
                sc = scores[T % 2][:]
                e2 = ec2[T % 2][:]          # [P, G*U, 2] dense pairs
                es = esum[T % 2][:]
                # exp into column 0 (with per-group denominator accumulators)
                # then column 1: dense pair writes keep ACT's SBUF footprint
                # small (v3's 32-wide strided ACT writes stretched DVE ops).
                for g in range(G):
                    a.activation(
                        e2[:, g * U : (g + 1) * U, 0],
                        sc[:, g * U : (g + 1) * U],
                        Exp,
                        accum_out=es[:, g : g + 1],
                    )
                a.activation(e2[:, :, 1], sc[:, :], Exp)
                # 1/esum = exp(-ln(esum)) -- vector.reciprocal is [P,1]-only
                # (5 ops); Ln/Exp share one ACT table set (natural_log_exp)
                a.activation(lnes[:], es, mybir.ActivationFunctionType.Ln)
                a.activation(recip[T % 2][:], lnes[:], Exp, scale=-1.0)
                # expand the dense pairs to 8 d-columns here (ACT ~15% busy;
                # these writes are >=25%-line-dense, unlike v3's 32-wide ones)
                et = e8[T % 2][:]
                a.activation(et[:, :, 0:2], e2, Copy)
                a.activation(et[:, :, 2:4], et[:, :, 0:2], Copy)
                ins = a.activation(et[:, :, 4:8], et[:, :, 0:4], Copy)
                ins.then_inc(AS, 1)

    _audit_waits(nc)
    return nc


_NC_CACHE = {}


def _get_nc():
    key = (G,)
    if key not in _NC_CACHE:
        _NC_CACHE[key] = _build_nc()
    return _NC_CACHE[key]


# ---------------------------------------------------------------- host side
def _prep_core_inputs(current_user_embedding, similar_user_embedding,
                      current_item_embedding, mask):
    # Fold the additive -1e9 mask into one sim column per batch:
    #   sim'[b,s,u,d0] = sim[b,s,u,d0] + maskNEG[b,s,u] / user[b,d0]
    # with d0 = argmax|user[b,:]|, so dot(sim', user) = scores + maskNEG
    # exactly (to bf16 rounding of a ~1e9 term). exp underflows to +0 for
    # masked entries, so the garbage sim' column contributes 0 to the
    # weighted-value sum as well. Removes the mask words from the DMA
    # stream and the mask add from the DVE per-tile program.
    in_maps = []
    d0 = np.abs(current_user_embedding).argmax(axis=1)              # [B]
    ud0 = np.take_along_axis(current_user_embedding, d0[:, None], 1)[:, 0]  # [B]
    for c in range(NCORES):
        b0, b1 = c * BC, (c + 1) * BC
        pk = np.empty((ROWS, ROWW), dtype=BF16)
        sim = similar_user_embedding[b0:b1].astype(np.float32, copy=True)  # [BC,S,U,D]
        adj = np.where(mask[b0:b1], np.float32(NEG), np.float32(0.0))      # [BC,S,U]
        adj /= ud0[b0:b1, None, None]
        for i in range(BC):
            sim[i, :, :, d0[b0 + i]] += adj[i]
        pk[:, :UD] = sim.reshape(ROWS, UD).astype(BF16)
        pk[:, o_user : o_user + D] = np.broadcast_to(
            current_user_embedding[b0:b1, None, :].astype(BF16), (BC, S, D)
        ).reshape(ROWS, D)
        pk[:, o_item:] = current_item_embedding[b0:b1].reshape(ROWS, D).astype(BF16)
        # reorder rows (T,g,p) into the tile-block layout [NT*P, BLKW]:
        # per (tile, partition): [sim g=0..G-1 | user g=0..G-1 | item g=0..G-1]
        pk3 = pk.reshape(NT, G, P, ROWW)
        sim_b = pk3[:, :, :, :UD].transpose(0, 2, 1, 3).reshape(NT, P, G * UD)
        usr_b = pk3[:, :, :, o_user : o_user + D].transpose(0, 2, 1, 3).reshape(NT, P, G * D)
        itm_b = pk3[:, :, :, o_item:].transpose(0, 2, 1, 3).reshape(NT, P, G * D)
        pkb = np.concatenate([sim_b, usr_b, itm_b], axis=2).reshape(NT * P, BLKW)
        in_maps.append({"pk": np.ascontiguousarray(pkb)})
    return in_maps


def _run(inputs, trace=False):
    nc = _get_nc()
    in_maps = _prep_core_inputs(**inputs)
    res = run_bass_kernel_spmd(
        nc, in_maps, core_ids=list(range(NCORES)), trace=trace
    )
    out = np.empty((B, S, D), dtype=np.float32)
    for c in range(NCORES):
        out[c * BC : (c + 1) * BC] = res.results[c]["out"].reshape(BC, S, D)
    return out, res


def kernel(**inputs):
    out, _ = _run(inputs, trace=False)
    return out



# revision 38
# speedup vs baseline: 1.2834x; 1.2834x over previous
"""Trainium2 Bass kernel for masked similar-user attention.

Computation (per batch b, position s):
    scores[u] = dot(user[b], sim[b,s,u,:])        (u = 50 similar users, d = 32)
    scores    = where(mask, -1e9, scores)
    attn      = softmax(scores)
    out[s]    = sum_u attn[u] * sim[b,s,u,:] + item[b,s]

Sharding: pure data parallel over batch (B=512 -> 64 per core, 8 cores).

Final version: 397us (v2 baseline 502us). Evolution, all trace-driven:
  * v3 (505us, reverted): mul2 (sim * e broadcast) ran in DVE 1x mode in
    v2 because its broadcast operand had innermost stride 0 (2x_1p needs
    stride +-1 on EVERY operand): 1883ns vs 1040ns for same-size mul1.
    v3 built a fully-expanded e32[.,u,d] on ACT with 32-wide strided
    writes: mul2 did hit 2x, but ACT went 50% busy and the SBUF
    contention inflated EVERY DVE op by 300-600ns -- a complete wash.
  * v4 (460us): expansion moved to dense single-src 4x-mode DVE copies;
    mask folded into one sim column host-side (d0 = argmax|user[b,:]|,
    sim[..,d0] += -1e9/user[b,d0], so the score dot-product produces
    scores+mask directly and exp underflows to exactly 0 for masked
    entries -- kills the per-tile mask add and shrinks the row to 1664
    words); G=5 (640-row tiles).
  * v5 (433us): tile-block DRAM layout (per tile/partition:
    [sim g=0..4 | user g=0..4 | item g=0..4]) makes the G sim groups one
    contiguous [P,G*U,D] region, so mul2 and the +item tail span all
    groups in single instructions (per-op fixed cost ~100-300ns; v4 ran
    36 DVE ops/tile).
  * v6 (420us): 1/esum on ACT as exp(-ln(esum)) (Ln+Exp share one table
    set), tail = one broadcast-mul + one add, replacing 5x [P,1]
    v.reciprocal + 5x per-group scalar_tensor_tensor.
  * v7/v8 (412 -> 397us): expansion narrowed to e8 (mul2 as 4 ops over
    d-quarters, same cycles, one less doubling) and the 3 doubling
    copies moved to ACT (~15% busy; these writes are >=25%-line-dense,
    unlike v3's 6%-dense ones -- no contention this time).
  * Dead ends, measured: reciprocal_approx_fast and AluOpType.divide
    both rejected by this walrus ("ISA wrong length" / 's3s3d3_tt_valid_op');
    G=10+NPK=2 (506us -- 2-deep prefetch ring serializes DMA);
    PE offload of either einsum dies on layout (contraction is per-row;
    every transpose path needs a PSUM->SBUF evacuation at 1 elem/cyc
    that costs more than it saves).
  * Remaining state: DVE ~95% busy at its 2x-mode floor (all tensor ops
    packed-bf16 stride-1), ACT ~20%, DMA ~25% of 358GB/s/core.

Pipeline (per tile T of 128x5 rows; sems LD/ST/VA/AS/VB):
    SP : prologue loads 0-3; iter T: [wait VB>=T+1] store T, load T+4
    DVE: iter T: [wait LD] A(T): mul1 x5, fold_d x5 -> scores  .inc VA
         [wait AS>=T] [wait ST] B(T-1): mul2 x4 (e8 d-quarters),
                      fold_u x6, outw*recip_bc, +item -> outt  .inc VB
    ACT: iter T: [wait VA>=T+1] 5x exp(+esum accum) + exp -> ec2 pairs,
         ln/exp -> 1/esum, 3 doubling copies -> e8  .inc AS
"""

import sys

if "/opt/trn_rl_repo" not in sys.path:
    sys.path.insert(0, "/opt/trn_rl_repo")

import numpy as np
import ml_dtypes

import concourse.bass as bass
from concourse import mybir
from concourse.bass_utils import run_bass_kernel_spmd


def _install_ntff_hook_shim():
    """The container's antenv lacks axon_hooks; recreate it so
    run_bass_kernel_spmd(trace=True) can capture NTFF profiles through
    libaxon_pjrt.so (same ctypes path trn_boot uses)."""
    import contextlib
    import ctypes
    import types

    if "antenv.axon_hooks" in sys.modules:
        return
    so_path = "/opt/axon/libaxon_pjrt.so"
    try:
        lib = ctypes.CDLL(so_path)
    except OSError:
        return
    if not hasattr(lib, "axon_start_nrt_profile"):
        return
    lib.axon_start_nrt_profile.argtypes = [
        ctypes.POINTER(ctypes.c_int64),
        ctypes.c_size_t,
    ]
    lib.axon_start_nrt_profile.restype = ctypes.c_int64
    lib.axon_stop_nrt_profile.argtypes = [ctypes.c_char_p]
    lib.axon_stop_nrt_profile.restype = ctypes.c_int64

    @contextlib.contextmanager
    def _hook(output_dir, device_ids):
        import jax

        jax.devices()
        if device_ids:
            ids = (ctypes.c_int64 * len(device_ids))(*device_ids)
            rc = lib.axon_start_nrt_profile(ids, len(device_ids))
        else:
            rc = lib.axon_start_nrt_profile(None, 0)
        if rc != 0:
            raise RuntimeError(f"axon_start_nrt_profile rc={rc}")
        try:
            yield
        finally:
            n = lib.axon_stop_nrt_profile(str(output_dir).encode())
            print(f"ntff profile: {n} file(s) written to {output_dir}")

    mod = types.ModuleType("antenv.axon_hooks")
    mod.get_axon_ntff_profile_hook = lambda: _hook
    mod.set_axon_ntff_profile_hook = lambda h: None
    sys.modules["antenv.axon_hooks"] = mod


_install_ntff_hook_shim()

# ---------------------------------------------------------------- config
B, S, U, D = 512, 200, 50, 32
NCORES = 8
BC = B // NCORES            # batches per core = 64
ROWS = BC * S               # rows per core = 12800
P = 128                     # SBUF partitions
G = 5                       # row-groups of 128 per DMA tile
NT = ROWS // (P * G)        # outer tiles per core = 20
NEG = -1e9
NPK = 4                     # pkt ring depth

UD = U * D                  # 1600
ROWW = UD + D + D           # packed row width (bf16 words) = 1664 (mask folded into sim)
o_user, o_item = UD, UD + D
# tile-block layout: per (tile, partition) the DRAM holds
#   [ sim for g=0..G-1 (G*1600) | user g=0..G-1 (G*32) | item g=0..G-1 (G*32) ]
# so the G sim groups are one contiguous [P, G*U, D] region -> mul2/item ops
# span all groups in ONE instruction (per-op fixed cost is ~290ns; v4 spent
# ~40% of DVE time on 36 ops/tile of overhead).
BLKW = G * ROWW             # per-partition tile block words = 8320
o_ub = G * UD               # user block offset = 8000
o_ib = G * UD + G * D       # item block offset = 8160

BF16 = np.dtype(ml_dtypes.bfloat16)


def _audit_waits(nc, max_waits=1):
    bad = []
    for blk in nc.m.functions[0].blocks:
        for ins in blk.instructions:
            si = ins.sync_info
            if si is not None and len(si.on_wait) > max_waits:
                bad.append((blk.name, ins.name, ins.opcode, len(si.on_wait)))
    if bad:
        msg = "\n".join(f"  {b}/{n} {o}: {k} waits" for b, n, o, k in bad)
        raise RuntimeError(f"instructions exceeding {max_waits} sync wait(s):\n{msg}")


# ---------------------------------------------------------------- kernel IR
def _build_nc():
    f32 = mybir.dt.float32
    bf16 = mybir.dt.bfloat16
    MUL = mybir.AluOpType.mult
    ADD = mybir.AluOpType.add
    nc = bass.Bass()

    pk_d = nc.dram_tensor("pk", [NT * P, BLKW], bf16, kind="ExternalInput")
    out_d = nc.dram_tensor("out", [ROWS, D], f32, kind="ExternalOutput")

    pk_v = pk_d[:].rearrange("(T p) f -> T p f", p=P)
    out_v = out_d[:].rearrange("(T g p) f -> T p g f", g=G, p=P)

    # SBUF buffers (all elementwise-op APs kept <= 3 dims: partition + 2 free)
    pkt = [nc.alloc_sbuf_tensor(f"pkt{i}", [P, BLKW], bf16) for i in range(NPK)]
    tmp = nc.alloc_sbuf_tensor("tmp", [P, G * U, D], bf16)    # mul1 out + fold_d scratch
    tmp2 = nc.alloc_sbuf_tensor("tmp2", [P, G * U, D], bf16)  # mul2 out + fold_u scratch
    scores = [nc.alloc_sbuf_tensor(f"scores{i}", [P, G * U], f32) for i in range(2)]
    # ec2[., u, 0:2] = exp(scores[., u]) written twice by ACT (dense pairs);
    # phase_b log2-doubles it into e32[., u, 0:32] with DVE 4x-mode copies so
    # that mul2's weight operand has innermost stride 1 -> 2x_1p mode
    # (a stride-0 broadcast operand forces 1x: 1883 vs 1040ns measured).
    # v3 built e32 on ACT instead: the strided ACT traffic inflated every
    # DVE op by 300-600ns (SBUF contention), a complete wash -- keep ACT light.
    ec2 = [nc.alloc_sbuf_tensor(f"ec2_{i}", [P, G * U, 2], bf16) for i in range(2)]
    e8 = [nc.alloc_sbuf_tensor(f"e8_{i}", [P, G * U, D // 4], bf16) for i in range(2)]
    esum = [nc.alloc_sbuf_tensor(f"esum{i}", [P, G], f32) for i in range(2)]
    lnes = nc.alloc_sbuf_tensor("lnes", [P, G], f32)
    recip = [nc.alloc_sbuf_tensor(f"recip{i}", [P, G], f32) for i in range(2)]
    outw = nc.alloc_sbuf_tensor("outw", [P, G, D], f32)
    outt = [nc.alloc_sbuf_tensor(f"outt{i}", [P, G * D], f32) for i in range(2)]

    LD = nc.alloc_semaphore("LD")
    ST = nc.alloc_semaphore("ST")
    VA = nc.alloc_semaphore("VA")
    AS = nc.alloc_semaphore("AS")
    VB = nc.alloc_semaphore("VB")


    def views(T):
        pkb = pkt[T % NPK][:]                                            # [P, BLKW]
        simblk = pkb[:, :o_ub].rearrange("p (u d) -> p u d", d=D)        # [P, G*U, D]
        usert = pkb[:, o_ub:o_ib].rearrange("p (g d) -> p g d", d=D)     # [P, G, D]
        itemt = pkb[:, o_ib:]                                            # [P, G*D]
        return simblk, usert, itemt

    with nc.Block() as blk:

        @blk.sync
        def _(sp):
            for T in range(min(NPK, NT)):
                sp.dma_start(out=pkt[T][:], in_=pk_v[T]).then_inc(LD, 16)
            for T in range(NT):
                sp.wait_ge(VB, T + 1)
                sp.dma_start(
                    out=out_v[T],
                    in_=outt[T % 2][:].rearrange("p (g w) -> p g w", g=G),
                ).then_inc(ST, 16)
                if T + NPK < NT:
                    sp.dma_start(
                        out=pkt[(T + NPK) % NPK][:], in_=pk_v[T + NPK]
                    ).then_inc(LD, 16)

        def phase_a(v, T):
            simblk, usert, _ = views(T)
            sc = scores[T % 2][:]
            # products (all-bf16 tensor_tensor -> 2x mode), per group (3-dim APs)
            for g in range(G):
                sim3 = simblk[:, g * U : (g + 1) * U, :]
                ub = usert[:, g, :].unsqueeze(1).broadcast_to([P, U, D])
                v.tensor_mul(tmp[:, g * U : (g + 1) * U, :], sim3, ub)
            # fold d: 32 -> 16 -> 8 -> 4 -> 2 (in place, all-bf16 2x)
            for k in (16, 8, 4, 2):
                v.tensor_add(tmp[:, :, 0:k], tmp[:, :, 0:k], tmp[:, :, k : 2 * k])
            # 2 -> 1, f32 out. The -1e9 mask is pre-folded into one sim column
            # host-side (sim[.,.,u,d0] += maskNEG/user[d0], d0 = argmax|user|),
            # so the dot product yields scores+mask directly -- no mask op.
            v.tensor_add(sc, tmp[:, :, 0], tmp[:, :, 1]).then_inc(VA, 1)

        def phase_b(v, T):
            simblk, _, itemt = views(T)
            et = e8[T % 2][:]
            # weighted values: tmp2 = sim * e (all-bf16, stride-1 -> 2x),
            # one op per d-quarter across ALL groups (sim groups contiguous)
            for j in range(4):
                v.tensor_mul(
                    tmp2[:, :, 8 * j : 8 * j + 8], simblk[:, :, 8 * j : 8 * j + 8], et
                )
            # fold u via the flat (u d) view: 50 -> 25 -> 16 -> 8 -> 4 -> 2 -> 1
            t2 = tmp2[:].rearrange("p (g u) d -> p g (u d)", g=G)
            v.tensor_add(t2[:, :, 0:800], t2[:, :, 0:800], t2[:, :, 800:1600])
            v.tensor_add(t2[:, :, 224:512], t2[:, :, 224:512], t2[:, :, 512:800])
            for k in (256, 128, 64):
                v.tensor_add(t2[:, :, 0:k], t2[:, :, 0:k], t2[:, :, k : 2 * k])
            v.tensor_add(outw[:], t2[:, :, 0:32], t2[:, :, 32:64])
            # out = outw * (1/esum) + item: recip comes from ACT as
            # exp(-ln(esum)) so the tail is 2 whole-tile ops instead of
            # 5x [P,1] reciprocal + 5x per-group stt (AluOpType.divide in
            # tensor_tensor fails 's3s3d3_tt_valid_op').
            rx = recip[T % 2][:].unsqueeze(2).broadcast_to([P, G, D])
            ot = outt[T % 2][:]
            v.tensor_mul(ot.rearrange("p (g w) -> p g w", g=G), outw[:], rx)
            ins = v.tensor_add(ot, ot, itemt)
            ins.then_inc(VB, 1)

        @blk.vector
        def _(v):
            for T in range(NT):
                v.wait_ge(LD, 16 * (T + 1))
                phase_a(v, T)
                if T >= 1:
                    v.wait_ge(AS, T)
                    if T >= 3:
                        v.wait_ge(ST, 16 * (T - 2))
                    phase_b(v, T - 1)
            v.wait_ge(AS, NT)
            v.wait_ge(ST, 16 * (NT - 2))
            phase_b(v, NT - 1)

        @blk.scalar
        def _(a):
            Exp = mybir.ActivationFunctionType.Exp
            Copy = mybir.ActivationFunctionType.Copy
            for T in range(NT):
                a.wait_ge(VA, T + 1)
                sc = scores[T % 2][:]
                e2 = ec2[T % 2][:]          # [P, G*U, 2] dense pairs
                es = esum[T % 2][:]
                # exp into column 0 (with per-group denominator accumulators)
                # then column 1: dense pair writes keep ACT's SBUF footprint
                # small (v3's 32-wide strided ACT writes stretched DVE ops).
                for g in range(G):
                    a.activation(
                        e2[:, g * U : (g + 1) * U, 0],
                        sc[:, g * U : (g + 1) * U],
                        Exp,
                        accum_out=es[:, g : g + 1],
                    )
                a.activation(e2[:, :, 1], sc[:, :], Exp)
                # 1/esum = exp(-ln(esum)) -- vector.reciprocal is [P,1]-only
                # (5 ops); Ln/Exp share one ACT table set (natural_log_exp)
                a.activation(lnes[:], es, mybir.ActivationFunctionType.Ln)
                a.activation(recip[T % 2][:], lnes[:], Exp, scale=-1.0)
                # expand the dense pairs to 8 d-columns here (ACT ~15% busy;
                # these writes are >=25%-line-dense, unlike v3's 32-wide ones)
                et = e8[T % 2][:]
                a.activation(et[:, :, 0:2], e2, Copy)
                a.activation(et[:, :, 2:4], et[:, :, 0:2], Copy)
                ins = a.activation(et[:, :, 4:8], et[:, :, 0:4], Copy)
                ins.then_inc(AS, 1)

    _audit_waits(nc)
    return nc


_NC_CACHE = {}


def _get_nc():
    key = (G,)
    if key not in _NC_CACHE:
        _NC_CACHE[key] = _build_nc()
    return _NC_CACHE[key]


# ---------------------------------------------------------------- host side
def _prep_core_inputs(current_user_embedding, similar_user_embedding,
                      current_item_embedding, mask):
    # Fold the additive -1e9 mask into one sim column per batch:
    #   sim'[b,s,u,d0] = sim[b,s,u,d0] + maskNEG[b,s,u] / user[b,d0]
    # with d0 = argmax|user[b,:]|, so dot(sim', user) = scores + maskNEG
    # exactly (to bf16 rounding of a ~1e9 term). exp underflows to +0 for
    # masked entries, so the garbage sim' column contributes 0 to the
    # weighted-value sum as well. Removes the mask words from the DMA
    # stream and the mask add from the DVE per-tile program.
    in_maps = []
    d0 = np.abs(current_user_embedding).argmax(axis=1)              # [B]
    ud0 = np.take_along_axis(current_user_embedding, d0[:, None], 1)[:, 0]  # [B]
    for c in range(NCORES):
        b0, b1 = c * BC, (c + 1) * BC
        pk = np.empty((ROWS, ROWW), dtype=BF16)
        sim = similar_user_embedding[b0:b1].astype(np.float32, copy=True)  # [BC,S,U,D]
        adj = np.where(mask[b0:b1], np.float32(NEG), np.float32(0.0))      # [BC,S,U]
        adj /= ud0[b0:b1, None, None]
        for i in range(BC):
            sim[i, :, :, d0[b0 + i]] += adj[i]
        pk[:, :UD] = sim.reshape(ROWS, UD).astype(BF16)
        pk[:, o_user : o_user + D] = np.broadcast_to(
            current_user_embedding[b0:b1, None, :].astype(BF16), (BC, S, D)
        ).reshape(ROWS, D)
        pk[:, o_item:] = current_item_embedding[b0:b1].reshape(ROWS, D).astype(BF16)
        # reorder rows (T,g,p) into the tile-block layout [NT*P, BLKW]:
        # per (tile, partition): [sim g=0..G-1 | user g=0..G-1 | item g=0..G-1]
        pk3 = pk.reshape(NT, G, P, ROWW)
        sim_b = pk3[:, :, :, :UD].transpose(0, 2, 1, 3).reshape(NT, P, G * UD)
        usr_b = pk3[:, :, :, o_user : o_user + D].transpose(0, 2, 1, 3).reshape(NT, P, G * D)
        itm_b = pk3[:, :, :, o_item:].transpose(0, 2, 1, 3).reshape(NT, P, G * D)
        pkb = np.concatenate([sim_b, usr_b, itm_b], axis=2).reshape(NT * P, BLKW)
        in_maps.append({"pk": np.ascontiguousarray(pkb)})
    return in_maps


def _run(inputs, trace=False):
    nc = _get_nc()
    in_maps = _prep_core_inputs(**inputs)
    res = run_bass_kernel_spmd(
        nc, in_maps, core_ids=list(range(NCORES)), trace=trace
    )
    out = np.empty((B, S, D), dtype=np.float32)
    for c in range(NCORES):
        out[c * BC : (c + 1) * BC] = res.results[c]["out"].reshape(BC, S, D)
    return out, res


def kernel(**inputs):
    out, _ = _run(inputs, trace=False)
    return out

